# revision 10
# baseline (speedup 1.0000x reference)
"""Causal self-attention with RoPE on 8 trn2 NeuronCores.

Sharding: heads 2r,2r+1 -> core r (both batches). w_attn column-sharded
(rows permuted even/odd per head so interleaved RoPE becomes rotate-half);
attention computed per-core in transposed [tk, tq] score layout; AllToAll
re-shards heads->sequence so each core runs the full output projection for
its own 512-token slice. Host only slices/permutes/casts inputs and
concatenates the 8 output slices. Matmul operands are bf16 (fp32 PSUM
accumulation); fp32 everywhere else.
"""

import math
import os
import sys
import tempfile

if "/opt/trn_rl_repo" not in sys.path:
    sys.path.insert(0, "/opt/trn_rl_repo")

import ml_dtypes
import numpy as np

import concourse.bacc as bacc
import concourse.bass as bass
import concourse.mybir as mybir
import concourse.tile as tile
from concourse.bass_utils import run_bass_kernel_spmd

B, T, D = 2, 2048, 2048
H, HD = 16, 128
NCORES = 8
HL = H // NCORES          # heads per core
BT = B * T
TS = 512                  # t supertile (psum bank width in fp32)
NTB = T // TS             # supertiles per batch
NE = D // 128             # e-chunks (contraction) in qkv
NKC = T // 128            # tk chunks per batch
FQKV = 3 * HL * HD        # 768 qkv features per core
FP = mybir.dt.float32
BF = mybir.dt.bfloat16
SCALE = 1.0 / math.sqrt(HD)

LAST_EXEC_NS = None
LAST_TRACE = None

_built = {}


def _install_ntff_shim():
    import types

    import antenv

    if "antenv.axon_hooks" not in sys.modules:
        mod = types.ModuleType("antenv.axon_hooks")
        _hook = [None]
        mod.set_axon_ntff_profile_hook = lambda h: _hook.__setitem__(0, h)
        mod.get_axon_ntff_profile_hook = lambda: _hook[0]
        sys.modules["antenv.axon_hooks"] = mod
        antenv.axon_hooks = mod
    from antenv.axon_hooks import (
        get_axon_ntff_profile_hook,
        set_axon_ntff_profile_hook,
    )

    if get_axon_ntff_profile_hook() is None:
        from trn_agent_boot.trn_boot import _ntff_profile_via_ctypes

        set_axon_ntff_profile_hook(_ntff_profile_via_ctypes("/opt/axon/libaxon_pjrt.so"))
    import concourse.bass_utils as bu

    bu.upload_artifacts = lambda tmpdir: f"local:{tmpdir}"


def _build():
    if "nc" in _built:
        return _built["nc"]
    nc = bacc.Bacc("TRN2", target_bir_lowering=False, debug=False, num_devices=NCORES)

    xT = nc.dram_tensor("xT", [D, BT], BF, kind="ExternalInput")
    wT = nc.dram_tensor("wT", [D, FQKV], BF, kind="ExternalInput")
    wpT = nc.dram_tensor("wpT", [D, D], BF, kind="ExternalInput")
    cs2 = nc.dram_tensor("cs2", [128, T], FP, kind="ExternalInput")
    sn2 = nc.dram_tensor("sn2", [128, T], FP, kind="ExternalInput")
    out_loc = nc.dram_tensor("out_loc", [TS, D], FP, kind="ExternalOutput")

    from contextlib import ExitStack

    ADD = mybir.AluOpType.add
    MUL = mybir.AluOpType.mult

    with tile.TileContext(nc) as tc:
        with ExitStack() as whole:
            dpool = whole.enter_context(tc.tile_pool(name="dram", bufs=1, space="DRAM"))
            a2a_in = [
                dpool.tile([NCORES, HD, TS], BF, name=f"a2a_in{hl}")
                for hl in range(HL)
            ]
            a2a_out = [
                dpool.tile([NCORES, HD, TS], BF, name=f"a2a_out{hl}")
                for hl in range(HL)
            ]
            # PSUM layout (16KB/partition): m0/m1 are double-bank [128,1024]
            # score/projection tiles; po0/po1 single-bank AV accumulators (also
            # phase-1 v psum + phase-3 banks); ps0/ps1 single-bank denominator
            # rows (also phase-3 banks).
            psum = whole.enter_context(tc.tile_pool(name="psum", bufs=1, space="PSUM"))
            smallp = whole.enter_context(tc.tile_pool(name="small", bufs=1))
            ones_f = smallp.tile([128, 1], FP, name="ones_f")
            nc.vector.memset(ones_f[:], 1.0)
            ones = smallp.tile([128, 1], BF, name="ones")
            nc.vector.tensor_copy(out=ones[:], in_=ones_f[:])
            # inclusive lower-triangular [128,128] mask (keep key_row p <= local col c)
            tri_f = smallp.tile([128, 128], FP, name="tri_f")
            nc.vector.memset(tri_f[:], 1.0)
            nc.gpsimd.affine_select(
                out=tri_f[:], in_=tri_f[:],
                pattern=[[1, 128]],
                compare_op=mybir.AluOpType.is_ge,
                fill=0.0, base=0, channel_multiplier=-1,
            )
            trimask = smallp.tile([128, 128], BF, name="trimask")
            nc.vector.tensor_copy(out=trimask[:], in_=tri_f[:])

            wq = whole.enter_context(tc.tile_pool(name="wq", bufs=1))
            w_sb = wq.tile([128, NE, FQKV], BF, name="w_sb")
            for e in range(NE):
                nc.sync.dma_start(
                    out=w_sb[:, e, :],
                    in_=wT[e * 128 : (e + 1) * 128, :],
                )

            tabs = whole.enter_context(tc.tile_pool(name="tabs", bufs=1))
            cs_sb = tabs.tile([128, T], FP, name="cs_sb")
            sn_sb = tabs.tile([128, T], FP, name="sn_sb")
            nc.sync.dma_start(out=cs_sb[:], in_=cs2[:])
            nc.sync.dma_start(out=sn_sb[:], in_=sn2[:])

            store = whole.enter_context(tc.tile_pool(name="store", bufs=1))
            xtp = whole.enter_context(tc.tile_pool(name="xt", bufs=2))
            work = whole.enter_context(tc.tile_pool(name="work", bufs=2))
            exps = whole.enter_context(tc.tile_pool(name="exps", bufs=3))
            spillp = whole.enter_context(tc.tile_pool(name="spill", bufs=1))
            accp = whole.enter_context(tc.tile_pool(name="accp", bufs=2))
            osbp = whole.enter_context(tc.tile_pool(name="osb", bufs=3))
            bcp = whole.enter_context(tc.tile_pool(name="bc", bufs=2))
            recp = whole.enter_context(tc.tile_pool(name="rec", bufs=2))
            yp = whole.enter_context(tc.tile_pool(name="yp", bufs=1))
            wpp = whole.enter_context(tc.tile_pool(name="wpp", bufs=12))
            outp = whole.enter_context(tc.tile_pool(name="outp", bufs=2))

            qrots, krots, v_alls = {}, {}, {}
            for b in range(B):
                qrot = [
                    store.tile([128, T], BF, tag=f"qrot{hl}_{b}", name=f"qrot{hl}_{b}")
                    for hl in range(HL)
                ]
                krot = [
                    store.tile([128, T], BF, tag=f"krot{hl}_{b}", name=f"krot{hl}_{b}")
                    for hl in range(HL)
                ]
                v_all = store.tile(
                    [128, HL, NKC, HD], BF, tag=f"v_all_{b}", name=f"v_all_{b}"
                )
                qrots[b], krots[b], v_alls[b] = qrot, krot, v_all

                # ---- phase 1: qkv projection + rope (per tb supertile) ----
                for tb in range(NTB):
                    toff = b * T + tb * TS
                    xt_t = xtp.tile([128, NE, TS], BF, tag="xt", name=f"xt_{b}_{tb}")
                    for c4 in range(4):
                        nc.gpsimd.dma_start(
                            out=xt_t[:, c4 * 4 : (c4 + 1) * 4, :],
                            in_=xT[c4 * 512 : (c4 + 1) * 512, toff : toff + TS].rearrange(
                                "(c p) t -> p c t", p=128
                            ),
                        )
                    qk0 = psum.tile([128, 2 * TS], FP, tag="m0", name=f"qk0_{b}_{tb}")
                    qk1 = psum.tile([128, 2 * TS], FP, tag="m1", name=f"qk1_{b}_{tb}")
                    pqk = [
                        qk0[:, 0:TS], qk0[:, TS : 2 * TS],
                        qk1[:, 0:TS], qk1[:, TS : 2 * TS],
                    ]
                    # one accumulation group per PSUM bank: a start=True matmul
                    # clears the has_written bits for its WHOLE bank, so two
                    # interleaved groups must not share a bank.
                    pv = [
                        psum.tile([128, TS], FP, tag=t, name=f"v{st}_{b}_{tb}")[
                            :, 0 : 2 * HD
                        ]
                        for st, t in enumerate(("po0", "po1", "ps0", "ps1"))
                    ]
                    # qk matmuls first; rope (vector) then overlaps the pv
                    # matmuls so m0/m1 are free before the next supertile.
                    for e in range(NE):
                        for g in range(4):
                            nc.tensor.matmul(
                                pqk[g],
                                lhsT=w_sb[:, e, g * 128 : (g + 1) * 128],
                                rhs=xt_t[:, e, :],
                                start=(e == 0),
                                stop=(e == NE - 1),
                                skip_group_check=True,
                            )
                    # rope: (even, odd) psum pairs -> rotated, assembled per head
                    csl = cs_sb[:, tb * TS : (tb + 1) * TS]
                    snl = sn_sb[:, tb * TS : (tb + 1) * TS]
                    for dst, pe_, po_ in ((qrot, pqk[0], pqk[1]), (krot, pqk[2], pqk[3])):
                        a_ = work.tile([128, TS], FP, tag="w0", name=f"a_{b}_{tb}")
                        b_ = work.tile([128, TS], FP, tag="w1", name=f"b_{b}_{tb}")
                        c_ = work.tile([128, TS], FP, tag="w2", name=f"c_{b}_{tb}")
                        d_ = work.tile([128, TS], FP, tag="w3", name=f"d_{b}_{tb}")
                        nc.vector.tensor_tensor(a_[:], pe_, csl, MUL)
                        nc.vector.tensor_tensor(b_[:], po_, snl, MUL)
                        nc.vector.tensor_tensor(c_[:], pe_, snl, MUL)
                        nc.vector.tensor_tensor(d_[:], po_, csl, MUL)
                        tsl = slice(tb * TS, (tb + 1) * TS)
                        for hl in range(HL):
                            hs = slice(hl * 64, (hl + 1) * 64)
                            nc.vector.tensor_tensor(
                                dst[hl][0:64, tsl], a_[hs, :], b_[hs, :],
                                mybir.AluOpType.subtract,
                            )
                            nc.vector.tensor_tensor(
                                dst[hl][64:128, tsl], c_[hs, :], d_[hs, :],
                                ADD,
                            )
                    for e in range(NE):
                        for st in range(4):
                            nc.tensor.matmul(
                                pv[st],
                                lhsT=xt_t[:, e, st * 128 : (st + 1) * 128],
                                rhs=w_sb[:, e, 512:768],
                                start=(e == 0),
                                stop=(e == NE - 1),
                                skip_group_check=True,
                            )
                    for st in range(4):
                        j = tb * 4 + st
                        for hl in range(HL):
                            nc.vector.tensor_copy(
                                out=v_all[:, hl, j, :],
                                in_=pv[st][:, hl * HD : (hl + 1) * HD],
                            )

            # ---- phase 2: attention ----
            # Scores in [tk, tq] layout. Off-diagonal key chunks are computed
            # in pairs packed into one [128,1024] psum tile (one exp each);
            # diagonal-supertile chunks are narrowed to their valid query
            # columns (widths 512/384/256/128, packed 896+384) and masked with
            # one precomputed [128,128] triangular mask on the vector engine.
            # Softmax denominators: bf16 vector accumulation of the (masked)
            # exp tiles + one [1,512] ones-matmul per supertile.
            exp_fn = mybir.ActivationFunctionType.Exp
            mc = [0]

            def attn_unit(hl, b, tb):
                qrot, krot, v_all = qrots[b], krots[b], v_alls[b]
                par = (b * NTB + tb) % 2
                po = psum.tile([128, TS], FP, tag=f"po{par}", name=f"o_{hl}_{b}_{tb}")
                tq0 = tb * TS
                acc = accp.tile([128, TS], BF, tag="acc", name=f"acc_{hl}_{b}_{tb}")
                first_av = [True]

                def av(dst_ap, j, rhs_ap, stop=False):
                    nc.tensor.matmul(
                        dst_ap,
                        lhsT=v_all[:, hl, j, :],
                        rhs=rhs_ap,
                        start=first_av[0],
                        stop=stop,
                        skip_group_check=True,
                    )
                    first_av[0] = False

                def score(dst_ap, j, q_lo, q_hi):
                    nc.tensor.matmul(
                        dst_ap,
                        lhsT=krot[hl][:, j * 128 : (j + 1) * 128],
                        rhs=qrot[hl][:, tq0 + q_lo : tq0 + q_hi],
                        start=True,
                        stop=True,
                        skip_group_check=True,
                    )

                # off-diagonal pairs
                for i in range(2 * tb):
                    j0, j1 = 2 * i, 2 * i + 1
                    sp = psum.tile(
                        [128, 2 * TS], FP, tag=f"m{mc[0] % 2}",
                        name=f"s_{hl}_{b}_{tb}_{i}",
                    )
                    mc[0] += 1
                    score(sp[:, 0:TS], j0, 0, TS)
                    score(sp[:, TS : 2 * TS], j1, 0, TS)
                    ex = exps.tile(
                        [128, 2 * TS], BF, tag="exp", name=f"e_{hl}_{b}_{tb}_{i}"
                    )
                    nc.scalar.activation(out=ex[:], in_=sp[:], func=exp_fn, scale=SCALE)
                    av(po[:], j0, ex[:, 0:TS])
                    av(po[:], j1, ex[:, TS : 2 * TS])
                    if i == 0:
                        nc.vector.tensor_tensor(acc[:], ex[:, 0:TS], ex[:, TS : 2 * TS], ADD)
                    else:
                        nc.vector.tensor_tensor(acc[:], acc[:], ex[:, 0:TS], ADD)
                        nc.vector.tensor_tensor(acc[:], acc[:], ex[:, TS : 2 * TS], ADD)

                # diagonal chunks jd..jd+3, valid widths 512/384/256/128
                jd = 4 * tb
                sp1 = psum.tile(
                    [128, 2 * TS], FP, tag=f"m{mc[0] % 2}", name=f"sd1_{hl}_{b}_{tb}"
                )
                mc[0] += 1
                score(sp1[:, 0:512], jd, 0, 512)
                score(sp1[:, 512:896], jd + 1, 128, 512)
                ex1 = exps.tile([128, 2 * TS], BF, tag="exp", name=f"ed1_{hl}_{b}_{tb}")
                nc.scalar.activation(
                    out=ex1[:, 0:896], in_=sp1[:, 0:896], func=exp_fn, scale=SCALE
                )
                nc.vector.tensor_tensor(ex1[:, 0:128], ex1[:, 0:128], trimask[:], MUL)
                nc.vector.tensor_tensor(ex1[:, 512:640], ex1[:, 512:640], trimask[:], MUL)
                av(po[:, 0:512], jd, ex1[:, 0:512])
                av(po[:, 128:512], jd + 1, ex1[:, 512:896])

                sp2 = psum.tile(
                    [128, 2 * TS], FP, tag=f"m{mc[0] % 2}", name=f"sd2_{hl}_{b}_{tb}"
                )
                mc[0] += 1
                score(sp2[:, 0:256], jd + 2, 256, 512)
                score(sp2[:, 256:384], jd + 3, 384, 512)
                ex2 = exps.tile([128, 2 * TS], BF, tag="exp", name=f"ed2_{hl}_{b}_{tb}")
                nc.scalar.activation(
                    out=ex2[:, 0:384], in_=sp2[:, 0:384], func=exp_fn, scale=SCALE
                )
                nc.vector.tensor_tensor(ex2[:, 0:128], ex2[:, 0:128], trimask[:], MUL)
                nc.vector.tensor_tensor(ex2[:, 256:384], ex2[:, 256:384], trimask[:], MUL)
                av(po[:, 256:512], jd + 2, ex2[:, 0:256])
                av(po[:, 384:512], jd + 3, ex2[:, 256:384], stop=True)

                if tb == 0:
                    nc.vector.tensor_copy(out=acc[:], in_=ex1[:, 0:512])
                else:
                    nc.vector.tensor_tensor(acc[:], acc[:], ex1[:, 0:512], ADD)
                nc.vector.tensor_tensor(acc[:, 128:512], acc[:, 128:512], ex1[:, 512:896], ADD)
                nc.vector.tensor_tensor(acc[:, 256:512], acc[:, 256:512], ex2[:, 0:256], ADD)
                nc.vector.tensor_tensor(acc[:, 384:512], acc[:, 384:512], ex2[:, 256:384], ADD)

                pss = psum.tile([128, TS], FP, tag=f"ps{par}", name=f"sm_{hl}_{b}_{tb}")
                nc.tensor.matmul(
                    pss[0:1, :], lhsT=ones[:], rhs=acc[:],
                    start=True, stop=True, skip_group_check=True,
                )
                rec = recp.tile([1, TS], FP, tag="rec", name=f"r_{hl}_{b}_{tb}")
                nc.vector.reciprocal_approx_fast(out=rec[:], in_=pss[0:1, :])
                bc = bcp.tile([128, TS], FP, tag="bc", name=f"bc_{hl}_{b}_{tb}")
                nc.gpsimd.partition_broadcast(bc[:], rec[:])
                osb = osbp.tile([128, TS], BF, tag="osb", name=f"ot_{hl}_{b}_{tb}")
                nc.vector.tensor_tensor(osb[:], po[:], bc[:], MUL)
                nc.sync.dma_start(out=a2a_in[hl][b * NTB + tb, :, :], in_=osb[:])

            y = [None] * NE

            def load_y(hl):
                for src in range(NCORES):
                    ci = HL * src + hl
                    yt = yp.tile([128, TS], BF, tag=f"y{ci}", name=f"y{ci}")
                    nc.sync.dma_start(out=yt[:], in_=a2a_out[hl][src, :, :])
                    y[ci] = yt

            for hl in range(HL):
                for b in range(B):
                    for tb in range(NTB):
                        attn_unit(hl, b, tb)
                        # hl=0 y loads are emitted mid-hl=1 so hl=1's osb DMAs
                        # are not queued behind them on the sync queue while
                        # the first all-to-all is still in flight.
                        if hl == 1 and b == 1 and tb == 1:
                            load_y(0)
                nc.gpsimd.collective_compute(
                    "AllToAll",
                    mybir.AluOpType.bypass,
                    replica_groups=[list(range(NCORES))],
                    ins=[a2a_in[hl][:].opt()],
                    outs=[a2a_out[hl][:].opt()],
                )

            # ---- phase 3: output projection for the local 512-token slice ----
            # Split by head parity: even-ci partial sums for dq 0/1 run while
            # the second all-to-all is in flight; odd ci finish them, then
            # dq 2/3 run with all ci.
            evens = list(range(0, NE, 2))
            odds = list(range(1, NE, 2))

            def pp_tiles(dq, idx):
                if dq % 2 == 0:
                    big0 = psum.tile([128, 2 * TS], FP, tag="m0", name=f"ppa_{idx}")
                    big1 = psum.tile([128, 2 * TS], FP, tag="m1", name=f"ppb_{idx}")
                    return [
                        big0[:, 0:TS], big0[:, TS : 2 * TS],
                        big1[:, 0:TS], big1[:, TS : 2 * TS],
                    ]
                return [
                    psum.tile([128, TS], FP, tag=t, name=f"pp{t}_{idx}")[:]
                    for t in ("po0", "po1", "ps0", "ps1")
                ]

            def proj(dq, pp, cis, first, last):
                for k, ci in enumerate(cis):
                    wpt = wpp.tile([128, TS], BF, tag="wp", name=f"wp_{dq}_{ci}")
                    nc.sync.dma_start(
                        out=wpt[:],
                        in_=wpT[ci * 128 : (ci + 1) * 128, dq * TS : (dq + 1) * TS],
                    )
                    for t2 in range(4):
                        nc.tensor.matmul(
                            pp[t2],
                            lhsT=y[ci][:, t2 * 128 : (t2 + 1) * 128],
                            rhs=wpt[:],
                            start=(first and k == 0),
                            stop=(last and k == len(cis) - 1),
                            skip_group_check=True,
                        )

            def write_out(dq, pp):
                for t2 in range(4):
                    ob = outp.tile([128, TS], FP, tag="ob", name=f"ob_{dq}_{t2}")
                    nc.vector.tensor_copy(out=ob[:], in_=pp[t2])
                    nc.sync.dma_start(
                        out=out_loc[t2 * 128 : (t2 + 1) * 128, dq * TS : (dq + 1) * TS],
                        in_=ob[:],
                    )

            # A1: dq0/1 even-ci complete groups, spilled to SBUF to free banks.
            spills = {}
            for dq in (0, 1):
                pp = pp_tiles(dq, f"A{dq}")
                proj(dq, pp, evens, first=True, last=True)
                sp_t = []
                for t2 in range(4):
                    s = spillp.tile([128, TS], FP, tag=f"sp{dq}_{t2}", name=f"sp_{dq}_{t2}")
                    nc.vector.tensor_copy(out=s[:], in_=pp[t2])
                    sp_t.append(s)
                spills[dq] = sp_t
            # A2: dq2/3 even-ci, psum-resident open groups (run during a2a1)
            ppC = {}
            for dq in (2, 3):
                ppC[dq] = pp_tiles(dq, f"B{dq}")
                proj(dq, ppC[dq], evens, first=True, last=False)
            load_y(1)
            # B2: dq2/3 odd-ci finish + drain
            for dq in (2, 3):
                proj(dq, ppC[dq], odds, first=False, last=True)
                write_out(dq, ppC[dq])
            # B1: dq0/1 odd-ci fresh groups, t2-major for a staggered tail;
            # output = fresh odd sums + spilled even partials
            for dq in (0, 1):
                pp = pp_tiles(dq, f"D{dq}")
                wpts = []
                for ci in odds:
                    wpt = wpp.tile([128, TS], BF, tag="wp", name=f"wpd_{dq}_{ci}")
                    nc.sync.dma_start(
                        out=wpt[:],
                        in_=wpT[ci * 128 : (ci + 1) * 128, dq * TS : (dq + 1) * TS],
                    )
                    wpts.append(wpt)
                for t2 in range(4):
                    for k, ci in enumerate(odds):
                        nc.tensor.matmul(
                            pp[t2],
                            lhsT=y[ci][:, t2 * 128 : (t2 + 1) * 128],
                            rhs=wpts[k][:],
                            start=(k == 0),
                            stop=(k == len(odds) - 1),
                            skip_group_check=True,
                        )
                    ob = outp.tile([128, TS], FP, tag="ob", name=f"obd_{dq}_{t2}")
                    nc.vector.tensor_tensor(ob[:], pp[t2], spills[dq][t2][:], ADD)
                    nc.sync.dma_start(
                        out=out_loc[t2 * 128 : (t2 + 1) * 128, dq * TS : (dq + 1) * TS],
                        in_=ob[:],
                    )

    nc.compile()
    _built["nc"] = nc
    return nc


def _host_prep(x, w_attn, w_proj):
    bf = ml_dtypes.bfloat16
    x2 = np.ascontiguousarray(x.reshape(BT, D).T.astype(bf))  # [D, BT] e-major
    wpT_full = np.ascontiguousarray(w_proj.T.astype(bf))      # [c, d]

    inv = 1.0 / (10000.0 ** (np.arange(0, HD, 2, dtype=np.float32) / HD))
    t = np.arange(T, dtype=np.float32)
    fr = np.outer(t, inv)                          # [T, 64]
    cosT = np.cos(fr).T.astype(np.float32)         # [64, T]
    sinT = np.sin(fr).T.astype(np.float32)
    cs2v = np.ascontiguousarray(np.vstack([cosT, cosT]))
    sn2v = np.ascontiguousarray(np.vstack([sinT, sinT]))

    perm = np.concatenate([np.arange(0, HD, 2), np.arange(1, HD, 2)])
    in_maps = []
    for r in range(NCORES):
        h0, h1 = HL * r, HL * r + 1
        rows = []
        for off in (0, D):  # q block then k block
            rows += [off + h0 * HD + perm[:64], off + h1 * HD + perm[:64]]
            rows += [off + h0 * HD + perm[64:], off + h1 * HD + perm[64:]]
        rows += [2 * D + h0 * HD + np.arange(HD), 2 * D + h1 * HD + np.arange(HD)]
        w_c = w_attn[np.concatenate(rows)]         # [768, D]
        wT_c = np.ascontiguousarray(w_c.T.astype(bf))  # [D, 768]
        in_maps.append(
            {"xT": x2, "wT": wT_c, "wpT": wpT_full, "cs2": cs2v, "sn2": sn2v}
        )
    return in_maps


def kernel(x, w_attn, w_proj):
    global LAST_EXEC_NS, LAST_TRACE
    x = np.asarray(x, dtype=np.float32)
    w_attn = np.asarray(w_attn, dtype=np.float32)
    w_proj = np.asarray(w_proj, dtype=np.float32)

    trace = os.environ.get("KERNEL_TRACE") == "1"
    if trace:
        _install_ntff_shim()

    nc = _build()
    in_maps = _host_prep(x, w_attn, w_proj)
    kw = {}
    if trace:
        tmpdir = os.environ.get("KERNEL_TRACE_DIR") or tempfile.mkdtemp(prefix="ktrace_")
        kw = dict(trace=True, tmpdir=tmpdir)
        LAST_TRACE = tmpdir
    res = run_bass_kernel_spmd(nc, in_maps, list(range(NCORES)), **kw)
    LAST_EXEC_NS = res.exec_time_ns

    out = np.empty((B, T, D), dtype=np.float32)
    for r in range(NCORES):
        b, tb = divmod(r, NTB)
        out[b, tb * TS : (tb + 1) * TS, :] = res.results[r]["out_loc"]
    return out

